# revision 15
# baseline (speedup 1.0000x reference)
"""WaveNet-style causal dilated conv stack (nn_CausalTemporalConv1d) on 8 TRN2 cores.

Strategy: data-parallel over batch (B=8 -> 1 batch element per core), params
replicated. Per core, everything stays on-chip except a DRAM-streamed cond
projection. Matmuls run in float32r (TF32-like, ~1e-4 relerr, bf16 speed);
the residual stream / GLU are f32r/f32 on DVE+ACT.

Per-core pipeline:
  x0 = start_w @ inp + start_b                     (fp32 matmul, K=80)
  c  = cond_w @ cond                               (fp32 matmul; biases folded
                                                    into per-layer bias vecs)
  c -> DRAM scratch, streamed back per layer       (saves 64KB/partition SBUF)
  for l in 0..7, d=2^l:
    A = dilated_causal_conv(x, w_in[l])            (f32r matmuls into PSUM,
                                                    boundary via partial-column
                                                    accumulation)
    pregate = A_gate + c_gate                      (DVE)
    sig     = Sigmoid(pregate + b_gate)            (ACT, per-partition bias)
    t_lin   = (A_lin + b_lin) + c_lin              (DVE scalar_tensor_tensor)
    acts    = t_lin * sig                          (DVE, f32r out)
    R = rs_w[l] @ acts                             (f32r matmuls)
    x' = (R_lin + rs_b_lin) + x                    (DVE stt, f32r out, ping-pong)
    skip += R_skip                                 (DVE; rs skip-biases folded
                                                    into end bias on host)
  out = end_w @ skip + end_b_eff                   (f32r matmul + ACT bias)
"""
import numpy as np

import concourse.bacc as bacc
import concourse.mybir as mybir
from concourse.tile import TileContext
from concourse.bass_utils import run_bass_kernel_spmd

F32 = mybir.dt.float32
F32R = mybir.dt.float32r
AF = mybir.ActivationFunctionType
ALU = mybir.AluOpType

N_CORES = 8
L = 8
K = 3
C = 256


def build_module(T=4096, iters=1):
    """Build the SPMD per-core Bass module. T must be a multiple of 512."""
    TW = 512
    NT = T // TW
    PAD = 256  # left zero-pad on x buffers: covers max tap shift (K-1)*d = 256
    nc = bacc.Bacc()

    inp_d = nc.declare_dram_parameter("inp", [80, T], F32, isOutput=False)
    cond_d = nc.declare_dram_parameter("cond", [128, 2, T], F32, isOutput=False)
    w_start_d = nc.declare_dram_parameter("w_start", [80, 256], F32, isOutput=False)
    w_cond_d = nc.declare_dram_parameter("w_cond", [128, 2, 512], F32, isOutput=False)
    w_in_d = nc.declare_dram_parameter("w_in", [L, 128, K, 2, 512], F32, isOutput=False)
    w_rs_d = nc.declare_dram_parameter("w_rs", [L - 1, 128, 2, 512], F32, isOutput=False)
    w_rsl_d = nc.declare_dram_parameter("w_rsl", [128, 2, 256], F32, isOutput=False)
    w_end_d = nc.declare_dram_parameter("w_end", [128, 2, 80], F32, isOutput=False)
    biases_d = nc.declare_dram_parameter("biases", [128, 51], F32, isOutput=False)
    out_d = nc.declare_dram_parameter("out", [80, T], F32, isOutput=True)

    c_dram = nc.dram_tensor("c_scratch", [NT, 128, 4, TW], F32)

    def bias_col(sb, col, rows=128):
        return sb[0:rows, col:col + 1]

    with TileContext(nc) as tc:
        with tc.tile_pool(name="sb", bufs=1) as pool, \
             tc.tile_pool(name="ps", bufs=1, space="PSUM") as psum:

            # ---- Phase A: small weights + biases (outside timing loop) ----
            biases_sb = pool.tile([128, 51], F32, tag="biases")
            nc.sync.dma_start(out=biases_sb[:], in_=biases_d[:])

            w_start_sb = pool.tile([80, 256], F32, tag="w_start")
            nc.sync.dma_start(out=w_start_sb[:], in_=w_start_d[:])

            w_rsl_st = pool.tile([128, 2, 256], F32, tag="w_rsl_st")
            nc.sync.dma_start(out=w_rsl_st[:], in_=w_rsl_d[:])
            w_rsl_r = pool.tile([128, 2, 256], F32R, tag="w_rsl")
            nc.vector.tensor_copy(w_rsl_r[:], w_rsl_st[:])

            w_end_st = pool.tile([128, 2, 80], F32, tag="w_end_st")
            nc.sync.dma_start(out=w_end_st[:], in_=w_end_d[:])
            w_end_r = pool.tile([128, 2, 80], F32R, tag="w_end")
            nc.vector.tensor_copy(w_end_r[:], w_end_st[:])

            def body():
                # ---- Phase B: start conv (fp32), x0 -> xa ----
                inp_sb = pool.tile([80, T], F32, tag="skip", name="inp_sb")
                nc.sync.dma_start(out=inp_sb[:], in_=inp_d[:])
                zpad = pool.tile([128, 2, PAD], F32, tag="zpad")
                nc.vector.memset(zpad[:], 0.0)
                xa = pool.tile([128, 2, PAD + T], F32R, tag="xa")
                nc.vector.tensor_copy(xa[:, :, 0:PAD], zpad[:])

                for t in range(NT):
                    t0 = t * TW
                    p0 = psum.tile([128, 2, TW], F32, tag="Ag", name="p_start")
                    for cb in range(2):
                        nc.tensor.matmul(p0[:, cb, :],
                                         w_start_sb[:, cb * 128:(cb + 1) * 128],
                                         inp_sb[:, t0:t0 + TW],
                                         start=True, stop=True)
                        nc.vector.tensor_scalar_add(
                            xa[:, cb, PAD + t0:PAD + t0 + TW], p0[:, cb, :],
                            bias_col(biases_sb, 48 + cb))

                # ---- Phase C: cond conv (fp32) -> c_dram ----
                cond_sb = pool.tile([128, 2, T], F32, tag="xb", name="cond_sb")
                nc.sync.dma_start(out=cond_sb[:], in_=cond_d[:])
                w_cond_sb = pool.tile([128, 2, 512], F32, tag="wst", name="w_cond_sb")
                nc.sync.dma_start(out=w_cond_sb[:], in_=w_cond_d[:])

                for t in range(NT):
                    t0 = t * TW
                    ctile = pool.tile([128, 4, TW], F32, tag="ctile", name="c_out",
                                      bufs=2)
                    for half in range(2):  # 0: lin co-blocks 0-1, 1: gate 2-3
                        pc = psum.tile([128, 2, TW], F32,
                                       tag=("Al" if half == 0 else "Ag"),
                                       name="p_cond")
                        for cb2 in range(2):
                            co = half * 2 + cb2
                            for ch in range(2):
                                nc.tensor.matmul(
                                    pc[:, cb2, :],
                                    w_cond_sb[:, ch, co * 128:(co + 1) * 128],
                                    cond_sb[:, ch, t0:t0 + TW],
                                    start=(ch == 0), stop=(ch == 1))
                        nc.scalar.activation(ctile[:, half * 2:(half + 1) * 2, :],
                                             pc[:], AF.Copy)
                    nc.sync.dma_start(out=c_dram[t], in_=ctile[:])

                # ---- Phase D: layers ----
                xb = pool.tile([128, 2, PAD + T], F32R, tag="xb", name="xb")
                nc.vector.tensor_copy(xb[:, :, 0:PAD], zpad[:])
                skip = pool.tile([128, 2, T], F32R, tag="skip", name="skip")

                for l in range(L):
                    d = 2 ** l
                    w_st = pool.tile([128, K, 2, 512], F32, tag="wst", name="w_in_st")
                    nc.sync.dma_start(out=w_st[:], in_=w_in_d[l])
                    w_in_r = pool.tile([128, K, 2, 512], F32R, tag="w_in", bufs=2,
                                       name="w_in_r")
                    nc.vector.tensor_copy(w_in_r[:], w_st[:])
                    if l < L - 1:
                        w_rs_st = pool.tile([128, 2, 512], F32, tag="wst",
                                            name="w_rs_st")
                        nc.sync.dma_start(out=w_rs_st[:], in_=w_rs_d[l])
                        w_rs_r = pool.tile([128, 2, 512], F32R, tag="w_rs", bufs=2,
                                           name="w_rs_r")
                        nc.vector.tensor_copy(w_rs_r[:], w_rs_st[:])

                    xp, xn = (xa, xb) if l % 2 == 0 else (xb, xa)

                    for t in range(NT):
                        t0 = t * TW
                        ctile = pool.tile([128, 4, TW], F32, tag="ctile",
                                          name="c_in", bufs=2)
                        nc.sync.dma_start(out=ctile[:], in_=c_dram[t])

                        def conv_in(ps, co):
                            """dilated conv for out-channel block co into ps."""
                            # tap k reads x[:, t0 + (k-2)*d : +TW] (zero-padded)
                            for i, (k, ch) in enumerate(
                                    [(k, ch) for k in (2, 1, 0) for ch in range(2)]):
                                shift = (2 - k) * d
                                nc.tensor.matmul(
                                    ps,
                                    w_in_r[:, k, ch, co * 128:(co + 1) * 128],
                                    xp[:, ch, PAD + t0 - shift: PAD + t0 - shift + TW],
                                    start=(i == 0), stop=(i == 5))

                        Ag = psum.tile([128, 2, TW], F32, tag="Ag", name="Ag")
                        for cb2 in range(2):
                            conv_in(Ag[:, cb2, :], 2 + cb2)
                        pregate = pool.tile([128, 2, TW], F32, tag="pregate",
                                            bufs=2, name="pregate")
                        nc.vector.tensor_tensor(pregate[:], Ag[:],
                                                ctile[:, 2:4, :], op=ALU.add)
                        sig = pool.tile([128, 2, TW], F32, tag="sig", bufs=2,
                                        name="sig")
                        for cb in range(2):
                            nc.scalar.activation(sig[:, cb, :], pregate[:, cb, :],
                                                 AF.Sigmoid,
                                                 bias=bias_col(biases_sb, l * 6 + 2 + cb))

                        Al = psum.tile([128, 2, TW], F32, tag="Al", name="Al")
                        for cb2 in range(2):
                            conv_in(Al[:, cb2, :], cb2)
                        t_lin = pool.tile([128, 2, TW], F32, tag="t_lin", bufs=2,
                                          name="t_lin")
                        for cb in range(2):
                            nc.vector.scalar_tensor_tensor(
                                t_lin[:, cb, :], Al[:, cb, :],
                                bias_col(biases_sb, l * 6 + cb),
                                ctile[:, cb, :], op0=ALU.add, op1=ALU.add)
                        acts = pool.tile([128, 2, TW], F32R, tag="acts", bufs=2,
                                         name="acts")
                        nc.vector.tensor_tensor(acts[:], t_lin[:], sig[:],
                                                op=ALU.mult)

                        # ---- rs conv ----
                        if l < L - 1:
                            Rxy = psum.tile([128, 2, TW], F32, tag="Rxy", name="Rxy")
                            Rsk = psum.tile([128, 2, TW], F32, tag="Rsk", name="Rsk")
                            for co in range(4):
                                ps = (Rxy if co < 2 else Rsk)[:, co % 2, :]
                                for ch in range(2):
                                    nc.tensor.matmul(
                                        ps, w_rs_r[:, ch, co * 128:(co + 1) * 128],
                                        acts[:, ch, :],
                                        start=(ch == 0), stop=(ch == 1))
                            for cb in range(2):
                                nc.vector.scalar_tensor_tensor(
                                    xn[:, cb, PAD + t0:PAD + t0 + TW], Rxy[:, cb, :],
                                    bias_col(biases_sb, l * 6 + 4 + cb),
                                    xp[:, cb, PAD + t0:PAD + t0 + TW],
                                    op0=ALU.add, op1=ALU.add)
                            if l == 0:
                                nc.vector.tensor_copy(skip[:, :, t0:t0 + TW], Rsk[:])
                            else:
                                nc.vector.tensor_tensor(
                                    skip[:, :, t0:t0 + TW], skip[:, :, t0:t0 + TW],
                                    Rsk[:], op=ALU.add)
                        else:
                            Rsk = psum.tile([128, 2, TW], F32, tag="Rsk", name="Rsk")
                            for co in range(2):
                                for ch in range(2):
                                    nc.tensor.matmul(
                                        Rsk[:, co, :],
                                        w_rsl_r[:, ch, co * 128:(co + 1) * 128],
                                        acts[:, ch, :],
                                        start=(ch == 0), stop=(ch == 1))
                            nc.vector.tensor_tensor(
                                skip[:, :, t0:t0 + TW], skip[:, :, t0:t0 + TW],
                                Rsk[:], op=ALU.add)

                # ---- Phase E: end conv ----
                for t in range(NT):
                    t0 = t * TW
                    pe = psum.tile([128, 2, TW], F32, tag="Ag", name="p_end")
                    for ch in range(2):
                        nc.tensor.matmul(pe[0:80, 0, :], w_end_r[:, ch, :],
                                         skip[:, ch, t0:t0 + TW],
                                         start=(ch == 0), stop=(ch == 1))
                    o_sb = pool.tile([80, TW], F32, tag="ostage", bufs=2,
                                     name="o_sb")
                    nc.scalar.activation(o_sb[:], pe[0:80, 0, :], AF.Identity,
                                         bias=bias_col(biases_sb, 50, rows=80))
                    nc.sync.dma_start(out=out_d[:, t0:t0 + TW], in_=o_sb[:])

            if iters == 1:
                body()
            else:
                with tc.For_i(0, iters):
                    body()

    nc.finalize()
    return nc


def prep_inputs(inp, cond, start_w, start_b, cond_w, cond_b, in_w, in_b,
                rs_w, rs_b, rs_w_last, rs_b_last, end_w, end_b):
    """Host-side weight/bias re-layout. Returns per-core in_maps."""
    f32 = np.float32
    T = inp.shape[-1]
    b_total = (in_b + cond_b[None, :]).astype(f32)          # [L, 512]
    b_lin, b_gate = b_total[:, :C], b_total[:, C:]
    rs_lin_b = rs_b[:, :C].astype(f32)                      # [L-1, 256]
    skip_bias = rs_b[:, C:].sum(0) + rs_b_last              # [256]
    end_b_eff = (end_b + end_w[:, :, 0] @ skip_bias).astype(f32)

    biases = np.zeros((128, 51), f32)
    for l in range(L):
        for ch in range(2):
            biases[:, l * 6 + ch] = b_lin[l, ch * 128:(ch + 1) * 128]
            biases[:, l * 6 + 2 + ch] = b_gate[l, ch * 128:(ch + 1) * 128]
            if l < L - 1:
                biases[:, l * 6 + 4 + ch] = rs_lin_b[l, ch * 128:(ch + 1) * 128]
    biases[:, 48] = start_b[:128]
    biases[:, 49] = start_b[128:]
    biases[:80, 50] = end_b_eff

    w_start = np.ascontiguousarray(start_w[:, :, 0].T, dtype=f32)       # [80,256]
    w_cond = np.ascontiguousarray(
        cond_w[:, :, 0].T.reshape(2, 128, 512).transpose(1, 0, 2), dtype=f32)
    w_in = np.ascontiguousarray(
        in_w.transpose(0, 2, 3, 1).reshape(L, 2, 128, K, 512)
            .transpose(0, 2, 3, 1, 4), dtype=f32)                        # [L,128,K,2,512]
    w_rs = np.ascontiguousarray(
        rs_w[:, :, :, 0].transpose(0, 2, 1).reshape(L - 1, 2, 128, 512)
            .transpose(0, 2, 1, 3), dtype=f32)                           # [L-1,128,2,512]
    w_rsl = np.ascontiguousarray(
        rs_w_last[:, :, 0].T.reshape(2, 128, 256).transpose(1, 0, 2), dtype=f32)
    w_end = np.ascontiguousarray(
        end_w[:, :, 0].T.reshape(2, 128, 80).transpose(1, 0, 2), dtype=f32)

    shared = dict(w_start=w_start, w_cond=w_cond, w_in=w_in, w_rs=w_rs,
                  w_rsl=w_rsl, w_end=w_end, biases=biases)
    in_maps = []
    for b in range(inp.shape[0]):
        m = dict(shared)
        m["inp"] = np.ascontiguousarray(inp[b], dtype=f32)
        m["cond"] = np.ascontiguousarray(
            cond[b].reshape(2, 128, T).transpose(1, 0, 2), dtype=f32)
        in_maps.append(m)
    return in_maps


_NC_CACHE = {}


def kernel(**inputs) -> np.ndarray:
    T = inputs["inp"].shape[-1]
    in_maps = prep_inputs(**inputs)
    key = (T, 1)
    if key not in _NC_CACHE:
        _NC_CACHE[key] = build_module(T=T, iters=1)
    nc = _NC_CACHE[key]
    res = run_bass_kernel_spmd(nc, in_maps, list(range(N_CORES))).results
    return np.stack([r["out"] for r in res]).astype(np.float32)
